# revision 8
# baseline (speedup 1.0000x reference)
"""Trainium2 Bass kernel for nn_MicroCoupledSuperNet (GNN message passing supernet).

Strategy (8-core SPMD, dst-node sharding):
  - Each core owns a contiguous range of destination nodes and all edges into them.
  - Per layer, both GCN (sym-normalized, self-loops) and SAGE-mean aggregations are
    computed with ONE matmul per 128-edge tile: gathered-source-rows^T @ E, where
    E in bf16 carries the per-edge weights (gcn_norm | 1/deg) into a combined
    [64 gcn cols | 64 sage cols] block of 64 destination nodes, accumulated in PSUM.
  - Source rows are fetched with dma_gather (int16 indices -> table split in two halves).
  - pre-MLP is deferred through the aggregation (A(xW) = (Ax)W), so layer 1 gathers
    straight from the x table; the dense stage fuses conv-mix into 3 matmuls per
    128-node block-pair, followed by a fused LayerNorm-mix + activation-mix chain.
  - h1 is exchanged between layers with an AllGather collective.
  - Sum-pool readout is a 0/1 matmul into per-core graph slots; host merges windows
    and adds post_b.
"""

import sys
import math
import dataclasses

import numpy as np

for _p in ("/opt/trn_rl_repo",):
    if _p not in sys.path:
        sys.path.insert(0, _p)

import ml_dtypes  # noqa: E402

BF16 = ml_dtypes.bfloat16

from concourse import bass, bacc, mybir, tile  # noqa: E402
from concourse.bass_utils import run_bass_kernel_spmd  # noqa: E402

P = 128          # SBUF partitions / edge-tile rows
BLK = 64         # destination nodes per aggregation block
H = 128          # hidden dim (== D_IN)
DOUT = 64
SBLK = 8         # aggregation blocks per superblock (scheduling unit)
GSLOTS = 128     # per-core graph slots for pooling
EPS = 1e-5
F32 = mybir.dt.float32
BF = mybir.dt.bfloat16
I16 = mybir.dt.int16


@dataclasses.dataclass
class Cfg:
    N: int
    E: int
    G: int
    cores: int
    half: int           # gather table split point (int16 index limit)
    nshard: int = 0
    nblk: int = 0
    npair: int = 0
    npad: int = 0
    nsb: int = 0

    def __post_init__(self):
        assert self.N % self.cores == 0
        self.nshard = self.N // self.cores
        self.nblk = math.ceil(self.nshard / BLK)
        if self.nblk % 2:
            self.nblk += 1  # keep whole pairs
        self.npair = self.nblk // 2
        self.npad = self.nblk * BLK
        self.nsb = math.ceil(self.nblk / SBLK)


def _softmax(v):
    v = np.asarray(v, np.float64)
    e = np.exp(v - v.max())
    return e / e.sum()


@dataclasses.dataclass
class Sched:
    """Static (cross-core-uniform) schedule + scalar constants."""
    T: np.ndarray            # [nblk, 2] tiles per (block, half)
    sb_nt: list              # per sb: (nt0, nt1)
    sb_idx_off: list         # per sb: (idx col offset half0, half1)
    sb_ecol: list            # per sb: E-stream col offset
    block_tiles: list        # per block: list of (half, slot_in_gbuf, eslot)
    idx_cols: int
    ecols: int
    nt0_max: int
    nt1_max: int
    et_max: int
    # scalar constants per layer
    wc: np.ndarray           # [L,2]
    wn: np.ndarray           # [L,2]
    wa: np.ndarray           # [L,3]
    have_bias1: bool
    have_bias2: bool
    have_lnb: list           # per layer: B row nonzero
    shard_rows: int          # real rows per shard (nshard)


def _build_schedule(cfg: Cfg, counts: np.ndarray) -> tuple:
    """counts: [cores, nblk, 2] edge counts. Returns tile schedule uniform across cores."""
    T = np.ceil(counts.max(axis=0) / P).astype(np.int64)  # [nblk, 2]
    sb_nt, sb_idx_off, sb_ecol, block_tiles = [], [], [], [None] * cfg.nblk
    idx_off = 0
    ecol = 0
    for sb in range(cfg.nsb):
        b0, b1 = sb * SBLK, min((sb + 1) * SBLK, cfg.nblk)
        nt0 = int(T[b0:b1, 0].sum())
        nt1 = int(T[b0:b1, 1].sum())
        sb_nt.append((nt0, nt1))
        sb_idx_off.append((idx_off, idx_off + nt0 * (P // 16)))
        idx_off += (nt0 + nt1) * (P // 16)
        sb_ecol.append(ecol)
        s0 = 0
        s1 = 0
        for b in range(b0, b1):
            tl = []
            for t in range(int(T[b, 0])):
                tl.append((0, s0, s0))
                s0 += 1
            for t in range(int(T[b, 1])):
                tl.append((1, s1, nt0 + s1))
                s1 += 1
            block_tiles[b] = tl
        ecol += (nt0 + nt1) * P
    nt0_max = max(nt for nt, _ in sb_nt)
    nt1_max = max(nt for _, nt in sb_nt)
    et_max = max(a + b for a, b in sb_nt)
    return T, sb_nt, sb_idx_off, sb_ecol, block_tiles, idx_off, ecol, nt0_max, nt1_max, et_max


def host_prep(inputs: dict, cfg: Cfg):
    """Numpy preprocessing: edge bucketing/tiling, E-matrix stream, index stream,
    combined weight matrices. Returns (sched, per-core in_maps data, combine info)."""
    x = np.asarray(inputs["x"], np.float32)
    ei = np.asarray(inputs["edge_index"])
    batch = np.asarray(inputs["batch"]).astype(np.int64)
    src = ei[0].astype(np.int64)
    dst = ei[1].astype(np.int64)
    N, E, G_N, C = cfg.N, cfg.E, cfg.G, cfg.cores
    ns = cfg.nshard

    deg_sl = np.bincount(dst, minlength=N).astype(np.float64) + 1.0  # with self loop
    dinv = 1.0 / np.sqrt(deg_sl)
    degn = np.maximum(np.bincount(dst, minlength=N), 1).astype(np.float64)

    # ---- per-core edge lists (with self-loop pseudo-edges) ----
    per_core = []
    counts = np.zeros((C, cfg.nblk, 2), np.int64)
    for c in range(C):
        lo, hi = c * ns, (c + 1) * ns
        m = (dst >= lo) & (dst < hi)
        es, ed = src[m], dst[m]
        dd = np.arange(lo, hi, dtype=np.int64)
        asrc = np.concatenate([es, dd])
        adst = np.concatenate([ed, dd])
        wg = np.concatenate([dinv[es] * dinv[ed], dinv[dd] ** 2])
        ws = np.concatenate([1.0 / degn[ed], np.zeros(ns)])
        dloc = adst - lo
        blk = dloc // BLK
        din = dloc % BLK
        hf = (asrc >= cfg.half).astype(np.int64)
        order = np.lexsort((hf, blk))
        asrc, wg, ws, blk, din, hf = (a[order] for a in (asrc, wg, ws, blk, din, hf))
        for b in range(cfg.nblk):
            mb = blk == b
            counts[c, b, 0] = int((mb & (hf == 0)).sum())
            counts[c, b, 1] = int((mb & (hf == 1)).sum())
        per_core.append((asrc, wg, ws, blk, din, hf))

    (T, sb_nt, sb_idx_off, sb_ecol, block_tiles, idx_cols, ecols,
     nt0_max, nt1_max, et_max) = _build_schedule(cfg, counts)

    # ---- pack per-core index + E streams ----
    data = []
    for c in range(C):
        asrc, wg, ws, blk, din, hf = per_core[c]
        # slot assignment: edges of (b, h) fill first counts[c,b,h] slots of its tiles
        idx_parts = []   # in gather-stream order (sb, half, block, tile)
        n_tiles_total = int(T.sum())
        Efull = np.zeros((n_tiles_total, P, P), np.float32)
        # global tile index per (b, h): tiles ordered sb-major, half-major, block
        tile_base = {}
        tix = 0
        for sb in range(cfg.nsb):
            b0, b1 = sb * SBLK, min((sb + 1) * SBLK, cfg.nblk)
            for hh in (0, 1):
                for b in range(b0, b1):
                    tile_base[(b, hh)] = tix
                    tix += int(T[b, hh])
        assert tix == n_tiles_total
        # scatter edges into tiles
        key = blk * 2 + hf
        order = np.argsort(key, kind="stable")
        asrc, wg, ws, blk, din, hf = (a[order] for a in (asrc, wg, ws, blk, din, hf))
        # position within (b, h) bucket
        pos = np.zeros(len(asrc), np.int64)
        start = 0
        for b in range(cfg.nblk):
            for hh in (0, 1):
                nbh = counts[c, b, hh]
                pos[start:start + nbh] = np.arange(nbh)
                start += nbh
        tno = np.array([tile_base[(int(b), int(h))] for b, h in zip(blk, hf)]) + pos // P
        prow = pos % P
        idxval = np.where(hf == 0, asrc, asrc - cfg.half)
        idx_stream = np.zeros((n_tiles_total, P), np.int64)
        idx_stream[tno, prow] = idxval
        Efull[tno, prow, din] = wg
        Efull[tno, prow, BLK + din] = ws
        # E stream partition-major [P, n_tiles*P]
        est = np.ascontiguousarray(
            Efull.transpose(1, 0, 2).reshape(P, n_tiles_total * P)).astype(BF16)
        # idx stream in 16-partition wrap, replicated to 128 partitions
        flat = idx_stream.reshape(-1)  # tile-major (matches gather stream order)
        assert len(flat) % 16 == 0
        wrapped = flat.reshape(-1, 16).T  # [16, ntot*8]
        idx16 = np.tile(wrapped, (8, 1)).astype(np.int16)  # [128, cols]
        assert idx16.shape[1] == idx_cols
        data.append({"est": est, "idx": idx16})

    # ---- pooling ----
    g_lo = []
    for c in range(C):
        lo = int(batch[c * ns])
        hi = int(batch[(c + 1) * ns - 1])
        span = hi - lo + 1
        assert span <= GSLOTS, f"graph span {span} exceeds {GSLOTS}"
        g_lo.append(lo)
        ep = np.zeros((cfg.npad, GSLOTS), np.float32)
        rows = np.arange(ns)
        ep[rows, batch[c * ns:(c + 1) * ns] - lo] = 1.0
        epm = np.ascontiguousarray(
            ep.reshape(cfg.npair, P, GSLOTS).transpose(1, 0, 2)
            .reshape(P, cfg.npair * GSLOTS)).astype(BF16)
        data[c]["epool"] = epm

    # ---- weights / constants ----
    pre_w = np.asarray(inputs["pre_w"], np.float64)
    pre_b = np.asarray(inputs["pre_b"], np.float64)
    post_w = np.asarray(inputs["post_w"], np.float64)
    post_b = np.asarray(inputs["post_b"], np.float64)
    gcn_w = np.asarray(inputs["gcn_w"], np.float64)
    gcn_b = np.asarray(inputs["gcn_b"], np.float64)
    sage_ws = np.asarray(inputs["sage_ws"], np.float64)
    sage_wn = np.asarray(inputs["sage_wn"], np.float64)
    ln_g = np.asarray(inputs["ln_g"], np.float64)
    ln_b = np.asarray(inputs["ln_b"], np.float64)
    a_conv = np.asarray(inputs["a_conv"], np.float64)
    a_norm = np.asarray(inputs["a_norm"], np.float64)
    a_act = np.asarray(inputs["a_act"], np.float64)

    wc = np.stack([_softmax(a_conv[l]) for l in range(2)])
    wn = np.stack([_softmax(a_norm[l]) for l in range(2)])
    wa = np.stack([_softmax(a_act[l]) for l in range(2)])

    Vg1 = pre_w @ (wc[0, 0] * gcn_w[0])
    VI1 = pre_w @ (wc[0, 1] * sage_ws[0])
    Vs1 = pre_w @ (wc[0, 1] * sage_wn[0])
    Vg2 = wc[1, 0] * gcn_w[1]
    VI2 = wc[1, 1] * sage_ws[1]
    Vs2 = wc[1, 1] * sage_wn[1]
    vm = np.stack([Vg1, VI1, Vs1, Vg2, VI2, Vs2]).astype(BF16)

    qg = wc[0, 0] * (pre_b @ gcn_w[0])
    qs = wc[0, 1] * (pre_b @ sage_wn[0])
    qc = wc[0, 0] * gcn_b[0] + wc[0, 1] * (pre_b @ sage_ws[0])
    bc2 = wc[1, 0] * gcn_b[1]
    qv = np.stack([qg, qs, qc, bc2]).astype(BF16)
    have_bias1 = bool(np.abs(qv[:3]).max() > 0)
    have_bias2 = bool(np.abs(bc2).max() > 0)

    # rs vectors (per-core, padded)
    rs_gcn_full = np.zeros(N)
    np.add.at(rs_gcn_full, dst, dinv[src])
    rs_gcn_full = dinv * rs_gcn_full + dinv ** 2
    rs_sage_full = (np.bincount(dst, minlength=N) > 0).astype(np.float64)
    for c in range(C):
        r = np.zeros((3, cfg.npad), np.float32)
        r[0, :ns] = rs_gcn_full[c * ns:(c + 1) * ns]
        r[1, :ns] = rs_sage_full[c * ns:(c + 1) * ns]
        r[2, :] = 1.0
        data[c]["rsv"] = r.astype(BF16)

    G1 = wn[0, 0] * ln_g[0]
    B1 = wn[0, 0] * ln_b[0]
    G2 = wn[1, 0] * ln_g[1]
    B2 = wn[1, 0] * ln_b[1]
    lnm = np.stack([np.tile(G1, (P, 1)), np.tile(B1, (P, 1)),
                    np.tile(G2, (P, 1)), np.tile(B2, (P, 1))]).astype(np.float32)
    have_lnb = [bool(np.abs(B1).max() > 0), bool(np.abs(B2).max() > 0)]

    xb = x.astype(BF16)  # global gather table
    for c in range(C):
        xs = np.zeros((cfg.npad, H), np.float32)
        xs[:ns] = x[c * ns:(c + 1) * ns]
        data[c]["xs"] = xs.astype(BF16)
        data[c]["xb"] = xb
        data[c]["vm"] = vm
        data[c]["qv"] = qv
        data[c]["lnm"] = lnm
        data[c]["pw"] = post_w.astype(BF16)
        data[c]["ident"] = np.eye(P, dtype=np.float32).astype(BF16)

    sched = Sched(T=T, sb_nt=sb_nt, sb_idx_off=sb_idx_off, sb_ecol=sb_ecol,
                  block_tiles=block_tiles, idx_cols=idx_cols, ecols=ecols,
                  nt0_max=nt0_max, nt1_max=nt1_max, et_max=et_max,
                  wc=wc, wn=wn, wa=wa,
                  have_bias1=have_bias1, have_bias2=have_bias2,
                  have_lnb=have_lnb, shard_rows=ns)
    combine = {"g_lo": g_lo, "post_b": post_b}
    return sched, data, combine


def build_program(cfg: Cfg, sched: Sched):
    nc = bacc.Bacc("TRN2", target_bir_lowering=False, debug=False,
                   enable_asserts=False, num_devices=cfg.cores)

    xb_d = nc.dram_tensor("xb", [cfg.N, H], BF, kind="ExternalInput")
    xs_d = nc.dram_tensor("xs", [cfg.npad, H], BF, kind="ExternalInput")
    idx_d = nc.dram_tensor("idx", [P, sched.idx_cols], I16, kind="ExternalInput")
    est_d = nc.dram_tensor("est", [P, sched.ecols], BF, kind="ExternalInput")
    epool_d = nc.dram_tensor("epool", [P, cfg.npair * GSLOTS], BF, kind="ExternalInput")
    vm_d = nc.dram_tensor("vm", [6, P, H], BF, kind="ExternalInput")
    qv_d = nc.dram_tensor("qv", [4, H], BF, kind="ExternalInput")
    rsv_d = nc.dram_tensor("rsv", [3, cfg.npad], BF, kind="ExternalInput")
    lnm_d = nc.dram_tensor("lnm", [4, P, H], F32, kind="ExternalInput")
    pw_d = nc.dram_tensor("pw", [H, DOUT], BF, kind="ExternalInput")
    ident_d = nc.dram_tensor("ident", [P, P], BF, kind="ExternalInput")
    out_d = nc.dram_tensor("out_part", [GSLOTS, DOUT], F32, kind="ExternalOutput")

    h1s_d = nc.dram_tensor("h1s", [cfg.nshard, H], BF)           # shard (collective in)
    h1f_d = nc.dram_tensor("h1f", [cfg.N, H], BF, addr_space="Shared")  # collective out

    ns = cfg.nshard
    L = 2

    with tile.TileContext(nc) as tc:
        with (
            tc.tile_pool(name="const", bufs=1) as cpool,
            tc.tile_pool(name="g0", bufs=2) as g0pool,
            tc.tile_pool(name="g1", bufs=2) as g1pool,
            tc.tile_pool(name="eb", bufs=2) as ebpool,
            tc.tile_pool(name="pairs", bufs=2 * SBLK + 4) as prpool,
            tc.tile_pool(name="z", bufs=2) as zpool,
            tc.tile_pool(name="lnt", bufs=2) as lnpool,
            tc.tile_pool(name="stat", bufs=4) as stpool,
            tc.tile_pool(name="xt", bufs=4) as xtpool,
            tc.tile_pool(name="small", bufs=4) as smpool,
            tc.tile_pool(name="ps_agg", bufs=2, space="PSUM") as ps_agg,
            tc.tile_pool(name="ps_dense", bufs=2, space="PSUM") as ps_dense,
            tc.tile_pool(name="ps_tr", bufs=2, space="PSUM") as ps_tr,
            tc.tile_pool(name="ps_pool", bufs=1, space="PSUM") as ps_pool,
        ):
            # ---------- resident constants ----------
            idx_t = cpool.tile([P, sched.idx_cols], I16)
            nc.sync.dma_start(out=idx_t[:], in_=idx_d.ap())
            epool_t = cpool.tile([P, cfg.npair * GSLOTS], BF)
            nc.sync.dma_start(out=epool_t[:], in_=epool_d.ap())
            vm_t = []
            for i in range(6):
                t = cpool.tile([P, H], BF, tag=f"vm{i}")
                nc.sync.dma_start(out=t[:], in_=vm_d.ap()[i])
                vm_t.append(t)
            ln_t = []
            for i in range(4):
                t = cpool.tile([P, H], F32, tag=f"ln{i}")
                nc.sync.dma_start(out=t[:], in_=lnm_d.ap()[i])
                ln_t.append(t)
            qv_t = []
            for i in range(4):
                t = cpool.tile([1, H], BF, tag=f"qv{i}")
                nc.sync.dma_start(out=t[:], in_=qv_d.ap()[i:i + 1, :])
                qv_t.append(t)
            rsv_t = []
            for i in range(3):
                t = cpool.tile([1, cfg.npad], BF, tag=f"rsv{i}")
                nc.sync.dma_start(out=t[:], in_=rsv_d.ap()[i:i + 1, :])
                rsv_t.append(t)
            pw_t = cpool.tile([H, DOUT], BF)
            nc.sync.dma_start(out=pw_t[:], in_=pw_d.ap())
            ident_t = cpool.tile([P, P], BF)
            nc.sync.dma_start(out=ident_t[:], in_=ident_d.ap())
            h1T_t = cpool.tile([P, cfg.npad], BF)      # feature-major h1 (own shard)
            h1loc_t = cpool.tile([P, cfg.npair * H], BF)  # node-major h1 (own shard)
            eps_t = cpool.tile([P, 1], F32)
            nc.vector.memset(eps_t[:], EPS)

            pool_psum = ps_pool.tile([GSLOTS, H], F32)

            def run_layer(l):
                wn1 = float(sched.wn[l, 1])
                ra = float(sched.wa[l, 0] + sched.wa[l, 2])
                ta = float(sched.wa[l, 1])
                ea = float(sched.wa[l, 2])
                g_rep = ln_t[2 * l]
                b_rep = ln_t[2 * l + 1]
                have_b = sched.have_lnb[l]
                bias_mm = sched.have_bias1 if l == 0 else sched.have_bias2
                table = xb_d.ap() if l == 0 else h1f_d.ap()
                tab_lo = table[0:cfg.half]
                tab_hi = table[cfg.half:cfg.N]

                for sb in range(cfg.nsb):
                    b0, b1 = sb * SBLK, min((sb + 1) * SBLK, cfg.nblk)
                    nt0, nt1 = sched.sb_nt[sb]
                    io0, io1 = sched.sb_idx_off[sb]
                    ec0 = sched.sb_ecol[sb]
                    npr = (b1 - b0) // 2
                    pr0 = b0 // 2

                    g0 = g0pool.tile([P, max(sched.nt0_max, 1) * P], BF, tag="g0")
                    g1 = g1pool.tile([P, max(sched.nt1_max, 1) * P], BF, tag="g1")
                    eb = ebpool.tile([P, sched.et_max * P], BF, tag="eb")
                    # dma_gather dies above 1024 indices/instruction (SWDGE
                    # descriptor-ring capacity) -> chunk into <=8-tile gathers.
                    GMAX = 8
                    for gbuf, ntn, ion, tabn in ((g0, nt0, io0, tab_lo),
                                                 (g1, nt1, io1, tab_hi)):
                        for t0 in range(0, ntn, GMAX):
                            tn = min(GMAX, ntn - t0)
                            nc.gpsimd.dma_gather(
                                out_ap=gbuf[:, t0 * P:(t0 + tn) * P]
                                .rearrange("p (t c) -> p t c", c=P),
                                in_ap=tabn,
                                idxs_ap=idx_t[:, ion + t0 * (P // 16):
                                              ion + (t0 + tn) * (P // 16)],
                                num_idxs=tn * P, num_idxs_reg=tn * P, elem_size=H)
                    nc.sync.dma_start(out=eb[:, :(nt0 + nt1) * P],
                                      in_=est_d.ap()[:, ec0:ec0 + (nt0 + nt1) * P])

                    gp = [None] * npr
                    sp = [None] * npr
                    for b in range(b0, b1):
                        tl = sched.block_tiles[b]
                        ps = ps_agg.tile([P, P], F32, tag="agg")
                        for k, (hf, slot, eslot) in enumerate(tl):
                            gsl = g0 if hf == 0 else g1
                            nc.tensor.matmul(
                                ps[:],
                                lhsT=gsl[:, slot * P:(slot + 1) * P],
                                rhs=eb[:, eslot * P:(eslot + 1) * P],
                                start=(k == 0), stop=(k == len(tl) - 1))
                        prl = (b - b0) // 2
                        side = b % 2
                        if side == 0:
                            gp[prl] = prpool.tile([P, P], BF, tag="gp", name=f"gp_{l}_{b}")
                            sp[prl] = prpool.tile([P, P], BF, tag="sp", name=f"sp_{l}_{b}")
                        nc.vector.tensor_copy(out=gp[prl][:, side * BLK:(side + 1) * BLK],
                                              in_=ps[:, 0:BLK])
                        nc.vector.tensor_copy(out=sp[prl][:, side * BLK:(side + 1) * BLK],
                                              in_=ps[:, BLK:2 * BLK])

                    z = zpool.tile([P, max(npr, 1) * H], F32, tag="z")
                    for prl in range(npr):
                        pr = pr0 + prl
                        if l == 0:
                            hT = xtpool.tile([P, P], BF, tag="xt")
                            nc.sync.dma_start(out=hT[:],
                                              in_=xs_d.ap()[pr * P:(pr + 1) * P, :],
                                              transpose=True)
                            hT_ap = hT[:]
                        else:
                            hT_ap = h1T_t[:, pr * P:(pr + 1) * P]
                        po = ps_dense.tile([P, H], F32, tag="dense")
                        nc.tensor.matmul(po[:], lhsT=gp[prl][:], rhs=vm_t[3 * l + 0][:],
                                         start=True, stop=False)
                        nc.tensor.matmul(po[:], lhsT=hT_ap, rhs=vm_t[3 * l + 1][:],
                                         start=False, stop=False)
                        nc.tensor.matmul(po[:], lhsT=sp[prl][:], rhs=vm_t[3 * l + 2][:],
                                         start=False, stop=not bias_mm)
                        if bias_mm:
                            if l == 0:
                                nc.tensor.matmul(po[:], lhsT=rsv_t[0][:, pr * P:(pr + 1) * P],
                                                 rhs=qv_t[0][:], start=False, stop=False)
                                nc.tensor.matmul(po[:], lhsT=rsv_t[1][:, pr * P:(pr + 1) * P],
                                                 rhs=qv_t[1][:], start=False, stop=False)
                                nc.tensor.matmul(po[:], lhsT=rsv_t[2][:, pr * P:(pr + 1) * P],
                                                 rhs=qv_t[2][:], start=False, stop=True)
                            else:
                                nc.tensor.matmul(po[:], lhsT=rsv_t[2][:, pr * P:(pr + 1) * P],
                                                 rhs=qv_t[3][:], start=False, stop=True)
                        nc.vector.tensor_copy(out=z[:, prl * H:(prl + 1) * H], in_=po[:])

                    # ---- fused LayerNorm-mix + activation-mix on [P, npr*H] ----
                    F = npr * H
                    z3 = z[:, :F].rearrange("p (g c) -> p g c", c=H)
                    mu = stpool.tile([P, max(npr, 1)], F32, tag="mu")
                    nc.vector.tensor_reduce(out=mu[:, :npr], in_=z3,
                                            axis=mybir.AxisListType.X, op=mybir.AluOpType.add)
                    nc.vector.tensor_scalar_mul(mu[:, :npr], mu[:, :npr], 1.0 / H)
                    zc = lnpool.tile([P, max(npr, 1) * H], F32, tag="zc")
                    nc.vector.tensor_tensor(out=zc[:, :F].rearrange("p (g c) -> p g c", c=H),
                                            in0=z3,
                                            in1=mu[:, :npr].to_broadcast([P, npr, H]),
                                            op=mybir.AluOpType.subtract)
                    sq = lnpool.tile([P, max(npr, 1) * H], F32, tag="sq")
                    nc.scalar.square(out=sq[:, :F], in_=zc[:, :F])
                    var = stpool.tile([P, max(npr, 1)], F32, tag="var")
                    nc.vector.tensor_reduce(out=var[:, :npr],
                                            in_=sq[:, :F].rearrange("p (g c) -> p g c", c=H),
                                            axis=mybir.AxisListType.X, op=mybir.AluOpType.add)
                    sd = stpool.tile([P, max(npr, 1)], F32, tag="sd")
                    nc.scalar.activation(out=sd[:, :npr], in_=var[:, :npr],
                                         func=mybir.ActivationFunctionType.Sqrt,
                                         bias=eps_t[:], scale=1.0 / H)
                    rsl = stpool.tile([P, max(npr, 1)], F32, tag="rsl")
                    nc.vector.reciprocal(out=rsl[:, :npr], in_=sd[:, :npr])
                    u = lnpool.tile([P, max(npr, 1) * H], F32, tag="u")
                    nc.vector.tensor_tensor(out=u[:, :F].rearrange("p (g c) -> p g c", c=H),
                                            in0=zc[:, :F].rearrange("p (g c) -> p g c", c=H),
                                            in1=rsl[:, :npr].to_broadcast([P, npr, H]),
                                            op=mybir.AluOpType.mult)
                    g_bc = dataclasses.replace(g_rep[:], ap=[g_rep[:].ap[0], [0, npr],
                                                             g_rep[:].ap[1]])
                    v = lnpool.tile([P, max(npr, 1) * H], F32, tag="v")
                    nc.vector.tensor_tensor(out=v[:, :F].rearrange("p (g c) -> p g c", c=H),
                                            in0=u[:, :F].rearrange("p (g c) -> p g c", c=H),
                                            in1=g_bc, op=mybir.AluOpType.mult)
                    w = zc  # reuse
                    nc.vector.tensor_scalar_mul(w[:, :F], z[:, :F], wn1)
                    hpre = u  # reuse
                    nc.vector.tensor_tensor(out=hpre[:, :F], in0=v[:, :F], in1=w[:, :F],
                                            op=mybir.AluOpType.add)
                    if have_b:
                        b_bc = dataclasses.replace(b_rep[:], ap=[b_rep[:].ap[0], [0, npr],
                                                                 b_rep[:].ap[1]])
                        nc.vector.tensor_tensor(
                            out=hpre[:, :F].rearrange("p (g c) -> p g c", c=H),
                            in0=hpre[:, :F].rearrange("p (g c) -> p g c", c=H),
                            in1=b_bc, op=mybir.AluOpType.add)
                    # activation mix: (wa0+wa2)*relu(x) + wa1*tanh(x) + wa2*exp(min(x,0)) - wa2
                    r_t = v  # reuse
                    nc.scalar.activation(out=r_t[:, :F], in_=hpre[:, :F],
                                         func=mybir.ActivationFunctionType.Relu, scale=ra)
                    th_t = sq  # reuse
                    nc.scalar.activation(out=th_t[:, :F], in_=hpre[:, :F],
                                         func=mybir.ActivationFunctionType.Tanh)
                    m_t = w  # reuse (zc)
                    nc.vector.tensor_scalar_min(m_t[:, :F], hpre[:, :F], 0.0)
                    e_t = z  # reuse z
                    nc.scalar.activation(out=e_t[:, :F], in_=m_t[:, :F],
                                         func=mybir.ActivationFunctionType.Exp)
                    nc.vector.tensor_scalar_mul(th_t[:, :F], th_t[:, :F], ta)
                    nc.vector.tensor_scalar(out=e_t[:, :F], in0=e_t[:, :F],
                                            scalar1=ea, scalar2=-ea,
                                            op0=mybir.AluOpType.mult,
                                            op1=mybir.AluOpType.add)
                    nc.vector.tensor_tensor(out=r_t[:, :F], in0=r_t[:, :F],
                                            in1=th_t[:, :F], op=mybir.AluOpType.add)
                    if l == 0:
                        hdst = h1loc_t[:, pr0 * H:pr0 * H + F]
                    else:
                        h2sb = lnpool.tile([P, max(npr, 1) * H], BF, tag="h2")
                        hdst = h2sb[:, :F]
                    nc.vector.tensor_tensor(out=hdst, in0=r_t[:, :F], in1=e_t[:, :F],
                                            op=mybir.AluOpType.add)

                    if l == 0:
                        for prl in range(npr):
                            pr = pr0 + prl
                            rows = min(P, ns - pr * P)
                            if rows > 0:
                                nc.sync.dma_start(
                                    out=h1s_d.ap()[pr * P:pr * P + rows, :],
                                    in_=h1loc_t[0:rows, pr * H:(pr + 1) * H])
                            pt = ps_tr.tile([P, P], BF, tag="tr")
                            nc.tensor.transpose(out=pt[:],
                                                in_=h1loc_t[:, pr * H:(pr + 1) * H],
                                                identity=ident_t[:])
                            nc.vector.tensor_copy(out=h1T_t[:, pr * P:(pr + 1) * P],
                                                  in_=pt[:])
                    else:
                        skip = lnpool.tile([P, max(npr, 1) * H], BF, tag="skip")
                        nc.vector.tensor_tensor(out=skip[:, :F],
                                                in0=h1loc_t[:, pr0 * H:pr0 * H + F],
                                                in1=hdst, op=mybir.AluOpType.add)
                        for prl in range(npr):
                            pr = pr0 + prl
                            nc.tensor.matmul(
                                pool_psum[:],
                                lhsT=epool_t[:, pr * GSLOTS:(pr + 1) * GSLOTS],
                                rhs=skip[:, prl * H:(prl + 1) * H],
                                start=(pr == 0), stop=(pr == cfg.npair - 1))

            run_layer(0)
            nc.gpsimd.collective_compute(
                "AllGather", mybir.AluOpType.bypass,
                replica_groups=[list(range(cfg.cores))],
                ins=[h1s_d.ap()], outs=[h1f_d.ap()])
            run_layer(1)

            # ---------- readout: pooled @ post_w ----------
            poolc = smpool.tile([GSLOTS, H], BF, tag="poolc")
            nc.vector.tensor_copy(out=poolc[:], in_=pool_psum[:])
            pt = ps_tr.tile([P, GSLOTS], BF, tag="tr")
            nc.tensor.transpose(out=pt[:], in_=poolc[:], identity=ident_t[:])
            ptc = smpool.tile([P, GSLOTS], BF, tag="ptc")
            nc.vector.tensor_copy(out=ptc[:], in_=pt[:])
            ops = ps_dense.tile([GSLOTS, DOUT], F32, tag="dense")
            nc.tensor.matmul(ops[:], lhsT=ptc[:], rhs=pw_t[:], start=True, stop=True)
            outc = smpool.tile([GSLOTS, DOUT], F32, tag="outc")
            nc.vector.tensor_copy(out=outc[:], in_=ops[:])
            nc.sync.dma_start(out=out_d.ap(), in_=outc[:])

    nc.compile()
    return nc


def _kernel_impl(inputs: dict, cfg: Cfg = None, trace: bool = False):
    if cfg is None:
        cfg = Cfg(N=50000, E=640000, G=500, cores=8, half=32768)
    sched, data, combine = host_prep(inputs, cfg)
    nc = build_program(cfg, sched)
    in_maps = [data[c] for c in range(cfg.cores)]
    res = run_bass_kernel_spmd(nc, in_maps, core_ids=list(range(cfg.cores)),
                               trace=trace)
    out = np.zeros((cfg.G, DOUT), np.float64)
    for c in range(cfg.cores):
        part = np.asarray(res.results[c]["out_part"], np.float64)
        lo = combine["g_lo"][c]
        hi = min(lo + GSLOTS, cfg.G)
        out[lo:hi] += part[:hi - lo]
    out += combine["post_b"]
    return out.astype(np.float32), res


def kernel(**inputs) -> np.ndarray:
    out, _ = _kernel_impl(inputs)
    return out


# revision 11
# speedup vs baseline: 1.6247x; 1.6247x over previous
"""Trainium2 Bass kernel for nn_MicroCoupledSuperNet (GNN message passing supernet).

Strategy (8-core SPMD, dst-node sharding):
  - Each core owns a contiguous range of destination nodes and all edges into them.
  - Per layer, both GCN (sym-normalized, self-loops) and SAGE-mean aggregations are
    computed with ONE matmul per 128-edge tile: gathered-source-rows^T @ E, where
    E in bf16 carries the per-edge weights (gcn_norm | 1/deg) into a combined
    [64 gcn cols | 64 sage cols] block of 64 destination nodes, accumulated in PSUM.
  - Source rows are fetched with dma_gather (int16 indices -> table split in two halves).
  - pre-MLP is deferred through the aggregation (A(xW) = (Ax)W), so layer 1 gathers
    straight from the x table; the dense stage fuses conv-mix into 3 matmuls per
    128-node block-pair, followed by a fused LayerNorm-mix + activation-mix chain.
  - h1 is exchanged between layers with an AllGather collective.
  - Sum-pool readout is a 0/1 matmul into per-core graph slots; host merges windows
    and adds post_b.
"""

import sys
import math
import dataclasses

import numpy as np

for _p in ("/opt/trn_rl_repo",):
    if _p not in sys.path:
        sys.path.insert(0, _p)

import ml_dtypes  # noqa: E402

BF16 = ml_dtypes.bfloat16

from concourse import bass, bacc, mybir, tile  # noqa: E402
from concourse.bass_utils import run_bass_kernel_spmd  # noqa: E402

P = 128          # SBUF partitions / edge-tile rows
BLK = 64         # destination nodes per aggregation block
H = 128          # hidden dim (== D_IN)
DOUT = 64
SBLK = 8         # aggregation blocks per superblock (scheduling unit)
GSLOTS = 128     # per-core graph slots for pooling
EPS = 1e-5
F32 = mybir.dt.float32
BF = mybir.dt.bfloat16
I16 = mybir.dt.int16


@dataclasses.dataclass
class Cfg:
    N: int
    E: int
    G: int
    cores: int
    half: int           # gather table split point (int16 index limit)
    nshard: int = 0
    nblk: int = 0
    npair: int = 0
    npad: int = 0
    nsb: int = 0

    def __post_init__(self):
        assert self.N % self.cores == 0
        self.nshard = self.N // self.cores
        self.nblk = math.ceil(self.nshard / BLK)
        if self.nblk % 2:
            self.nblk += 1  # keep whole pairs
        self.npair = self.nblk // 2
        self.npad = self.nblk * BLK
        self.nsb = math.ceil(self.nblk / SBLK)


def _softmax(v):
    v = np.asarray(v, np.float64)
    e = np.exp(v - v.max())
    return e / e.sum()


@dataclasses.dataclass
class Sched:
    """Static (cross-core-uniform) schedule + scalar constants."""
    T: np.ndarray            # [nblk, 2] tiles per (block, half)
    sb_nt: list              # per sb: (nt0, nt1)
    sb_idx_off: list         # per sb: (idx col offset half0, half1)
    sb_ecol: list            # per sb: E-stream col offset
    block_tiles: list        # per block: list of (half, slot_in_gbuf, eslot)
    idx_cols: int
    ecols: int
    nt0_max: int
    nt1_max: int
    et_max: int
    # scalar constants per layer
    wc: np.ndarray           # [L,2]
    wn: np.ndarray           # [L,2]
    wa: np.ndarray           # [L,3]
    have_bias1: bool
    have_bias2: bool
    have_lnb: list           # per layer: B row nonzero
    shard_rows: int          # real rows per shard (nshard)


def _build_schedule(cfg: Cfg, counts: np.ndarray) -> tuple:
    """counts: [cores, nblk, 2] edge counts. Returns tile schedule uniform across cores."""
    T = np.ceil(counts.max(axis=0) / P).astype(np.int64)  # [nblk, 2]
    sb_nt, sb_idx_off, sb_ecol, block_tiles = [], [], [], [None] * cfg.nblk
    idx_off = 0
    ecol = 0
    for sb in range(cfg.nsb):
        b0, b1 = sb * SBLK, min((sb + 1) * SBLK, cfg.nblk)
        nt0 = int(T[b0:b1, 0].sum())
        nt1 = int(T[b0:b1, 1].sum())
        sb_nt.append((nt0, nt1))
        sb_idx_off.append((idx_off, idx_off + nt0 * (P // 16)))
        idx_off += (nt0 + nt1) * (P // 16)
        sb_ecol.append(ecol)
        s0 = 0
        s1 = 0
        for b in range(b0, b1):
            tl = []
            for t in range(int(T[b, 0])):
                tl.append((0, s0, s0))
                s0 += 1
            for t in range(int(T[b, 1])):
                tl.append((1, s1, nt0 + s1))
                s1 += 1
            block_tiles[b] = tl
        ecol += (nt0 + nt1) * P
    nt0_max = max(nt for nt, _ in sb_nt)
    nt1_max = max(nt for _, nt in sb_nt)
    et_max = max(a + b for a, b in sb_nt)
    return T, sb_nt, sb_idx_off, sb_ecol, block_tiles, idx_off, ecol, nt0_max, nt1_max, et_max


def host_prep(inputs: dict, cfg: Cfg):
    """Numpy preprocessing: edge bucketing/tiling, E-matrix stream, index stream,
    combined weight matrices. Returns (sched, per-core in_maps data, combine info)."""
    x = np.asarray(inputs["x"], np.float32)
    ei = np.asarray(inputs["edge_index"])
    batch = np.asarray(inputs["batch"]).astype(np.int64)
    src = ei[0].astype(np.int64)
    dst = ei[1].astype(np.int64)
    N, E, G_N, C = cfg.N, cfg.E, cfg.G, cfg.cores
    ns = cfg.nshard

    deg_sl = np.bincount(dst, minlength=N).astype(np.float64) + 1.0  # with self loop
    dinv = 1.0 / np.sqrt(deg_sl)
    degn = np.maximum(np.bincount(dst, minlength=N), 1).astype(np.float64)

    # ---- per-core edge lists (with self-loop pseudo-edges) ----
    per_core = []
    counts = np.zeros((C, cfg.nblk, 2), np.int64)
    for c in range(C):
        lo, hi = c * ns, (c + 1) * ns
        m = (dst >= lo) & (dst < hi)
        es, ed = src[m], dst[m]
        dd = np.arange(lo, hi, dtype=np.int64)
        asrc = np.concatenate([es, dd])
        adst = np.concatenate([ed, dd])
        wg = np.concatenate([dinv[es] * dinv[ed], dinv[dd] ** 2])
        ws = np.concatenate([1.0 / degn[ed], np.zeros(ns)])
        dloc = adst - lo
        blk = dloc // BLK
        din = dloc % BLK
        hf = (asrc >= cfg.half).astype(np.int64)
        order = np.lexsort((hf, blk))
        asrc, wg, ws, blk, din, hf = (a[order] for a in (asrc, wg, ws, blk, din, hf))
        for b in range(cfg.nblk):
            mb = blk == b
            counts[c, b, 0] = int((mb & (hf == 0)).sum())
            counts[c, b, 1] = int((mb & (hf == 1)).sum())
        per_core.append((asrc, wg, ws, blk, din, hf))

    (T, sb_nt, sb_idx_off, sb_ecol, block_tiles, idx_cols, ecols,
     nt0_max, nt1_max, et_max) = _build_schedule(cfg, counts)

    # ---- pack per-core index + E streams ----
    data = []
    for c in range(C):
        asrc, wg, ws, blk, din, hf = per_core[c]
        # slot assignment: edges of (b, h) fill first counts[c,b,h] slots of its tiles
        idx_parts = []   # in gather-stream order (sb, half, block, tile)
        n_tiles_total = int(T.sum())
        Efull = np.zeros((n_tiles_total, P, P), np.float32)
        # global tile index per (b, h): tiles ordered sb-major, half-major, block
        tile_base = {}
        tix = 0
        for sb in range(cfg.nsb):
            b0, b1 = sb * SBLK, min((sb + 1) * SBLK, cfg.nblk)
            for hh in (0, 1):
                for b in range(b0, b1):
                    tile_base[(b, hh)] = tix
                    tix += int(T[b, hh])
        assert tix == n_tiles_total
        # scatter edges into tiles
        key = blk * 2 + hf
        order = np.argsort(key, kind="stable")
        asrc, wg, ws, blk, din, hf = (a[order] for a in (asrc, wg, ws, blk, din, hf))
        # position within (b, h) bucket
        pos = np.zeros(len(asrc), np.int64)
        start = 0
        for b in range(cfg.nblk):
            for hh in (0, 1):
                nbh = counts[c, b, hh]
                pos[start:start + nbh] = np.arange(nbh)
                start += nbh
        tno = np.array([tile_base[(int(b), int(h))] for b, h in zip(blk, hf)]) + pos // P
        prow = pos % P
        idxval = np.where(hf == 0, asrc, asrc - cfg.half)
        idx_stream = np.zeros((n_tiles_total, P), np.int64)
        idx_stream[tno, prow] = idxval
        Efull[tno, prow, din] = wg
        Efull[tno, prow, BLK + din] = ws
        # E stream partition-major [P, n_tiles*P]
        est = np.ascontiguousarray(
            Efull.transpose(1, 0, 2).reshape(P, n_tiles_total * P)).astype(BF16)
        # idx stream in 16-partition wrap, replicated to 128 partitions
        flat = idx_stream.reshape(-1)  # tile-major (matches gather stream order)
        assert len(flat) % 16 == 0
        wrapped = flat.reshape(-1, 16).T  # [16, ntot*8]
        idx16 = np.tile(wrapped, (8, 1)).astype(np.int16)  # [128, cols]
        assert idx16.shape[1] == idx_cols
        data.append({"est": est, "idx": idx16})

    # ---- pooling ----
    g_lo = []
    for c in range(C):
        lo = int(batch[c * ns])
        hi = int(batch[(c + 1) * ns - 1])
        span = hi - lo + 1
        assert span <= GSLOTS, f"graph span {span} exceeds {GSLOTS}"
        g_lo.append(lo)
        ep = np.zeros((cfg.npad, GSLOTS), np.float32)
        rows = np.arange(ns)
        ep[rows, batch[c * ns:(c + 1) * ns] - lo] = 1.0
        epm = np.ascontiguousarray(
            ep.reshape(cfg.npair, P, GSLOTS).transpose(1, 0, 2)
            .reshape(P, cfg.npair * GSLOTS)).astype(BF16)
        data[c]["epool"] = epm

    # ---- weights / constants ----
    pre_w = np.asarray(inputs["pre_w"], np.float64)
    pre_b = np.asarray(inputs["pre_b"], np.float64)
    post_w = np.asarray(inputs["post_w"], np.float64)
    post_b = np.asarray(inputs["post_b"], np.float64)
    gcn_w = np.asarray(inputs["gcn_w"], np.float64)
    gcn_b = np.asarray(inputs["gcn_b"], np.float64)
    sage_ws = np.asarray(inputs["sage_ws"], np.float64)
    sage_wn = np.asarray(inputs["sage_wn"], np.float64)
    ln_g = np.asarray(inputs["ln_g"], np.float64)
    ln_b = np.asarray(inputs["ln_b"], np.float64)
    a_conv = np.asarray(inputs["a_conv"], np.float64)
    a_norm = np.asarray(inputs["a_norm"], np.float64)
    a_act = np.asarray(inputs["a_act"], np.float64)

    wc = np.stack([_softmax(a_conv[l]) for l in range(2)])
    wn = np.stack([_softmax(a_norm[l]) for l in range(2)])
    wa = np.stack([_softmax(a_act[l]) for l in range(2)])

    Vg1 = pre_w @ (wc[0, 0] * gcn_w[0])
    VI1 = pre_w @ (wc[0, 1] * sage_ws[0])
    Vs1 = pre_w @ (wc[0, 1] * sage_wn[0])
    Vg2 = wc[1, 0] * gcn_w[1]
    VI2 = wc[1, 1] * sage_ws[1]
    Vs2 = wc[1, 1] * sage_wn[1]
    vm = np.stack([Vg1, VI1, Vs1, Vg2, VI2, Vs2]).astype(BF16)

    qg = wc[0, 0] * (pre_b @ gcn_w[0])
    qs = wc[0, 1] * (pre_b @ sage_wn[0])
    qc = wc[0, 0] * gcn_b[0] + wc[0, 1] * (pre_b @ sage_ws[0])
    bc2 = wc[1, 0] * gcn_b[1]
    qv = np.stack([qg, qs, qc, bc2]).astype(BF16)
    have_bias1 = bool(np.abs(qv[:3]).max() > 0)
    have_bias2 = bool(np.abs(bc2).max() > 0)

    # rs vectors (per-core, padded)
    rs_gcn_full = np.zeros(N)
    np.add.at(rs_gcn_full, dst, dinv[src])
    rs_gcn_full = dinv * rs_gcn_full + dinv ** 2
    rs_sage_full = (np.bincount(dst, minlength=N) > 0).astype(np.float64)
    for c in range(C):
        r = np.zeros((3, cfg.npad), np.float32)
        r[0, :ns] = rs_gcn_full[c * ns:(c + 1) * ns]
        r[1, :ns] = rs_sage_full[c * ns:(c + 1) * ns]
        r[2, :] = 1.0
        data[c]["rsv"] = r.astype(BF16)

    G1 = wn[0, 0] * ln_g[0]
    B1 = wn[0, 0] * ln_b[0]
    G2 = wn[1, 0] * ln_g[1]
    B2 = wn[1, 0] * ln_b[1]
    lnm = np.stack([np.tile(G1, (P, 1)), np.tile(B1, (P, 1)),
                    np.tile(G2, (P, 1)), np.tile(B2, (P, 1))]).astype(np.float32)
    have_lnb = [bool(np.abs(B1).max() > 0), bool(np.abs(B2).max() > 0)]

    xb = x.astype(BF16)  # global gather table
    for c in range(C):
        xs = np.zeros((cfg.npad, H), np.float32)
        xs[:ns] = x[c * ns:(c + 1) * ns]
        data[c]["xs"] = xs.astype(BF16)
        data[c]["xb"] = xb
        data[c]["vm"] = vm
        data[c]["qv"] = qv
        data[c]["lnm"] = lnm
        data[c]["pw"] = post_w.astype(BF16)
        data[c]["ident"] = np.eye(P, dtype=np.float32).astype(BF16)

    sched = Sched(T=T, sb_nt=sb_nt, sb_idx_off=sb_idx_off, sb_ecol=sb_ecol,
                  block_tiles=block_tiles, idx_cols=idx_cols, ecols=ecols,
                  nt0_max=nt0_max, nt1_max=nt1_max, et_max=et_max,
                  wc=wc, wn=wn, wa=wa,
                  have_bias1=have_bias1, have_bias2=have_bias2,
                  have_lnb=have_lnb, shard_rows=ns)
    combine = {"g_lo": g_lo, "post_b": post_b}
    return sched, data, combine


def build_program(cfg: Cfg, sched: Sched):
    nc = bacc.Bacc("TRN2", target_bir_lowering=False, debug=False,
                   enable_asserts=False, num_devices=cfg.cores,
                   num_swdge_queues=4)

    xb_d = nc.dram_tensor("xb", [cfg.N, H], BF, kind="ExternalInput")
    xs_d = nc.dram_tensor("xs", [cfg.npad, H], BF, kind="ExternalInput")
    idx_d = nc.dram_tensor("idx", [P, sched.idx_cols], I16, kind="ExternalInput")
    est_d = nc.dram_tensor("est", [P, sched.ecols], BF, kind="ExternalInput")
    epool_d = nc.dram_tensor("epool", [P, cfg.npair * GSLOTS], BF, kind="ExternalInput")
    vm_d = nc.dram_tensor("vm", [6, P, H], BF, kind="ExternalInput")
    qv_d = nc.dram_tensor("qv", [4, H], BF, kind="ExternalInput")
    rsv_d = nc.dram_tensor("rsv", [3, cfg.npad], BF, kind="ExternalInput")
    lnm_d = nc.dram_tensor("lnm", [4, P, H], F32, kind="ExternalInput")
    pw_d = nc.dram_tensor("pw", [H, DOUT], BF, kind="ExternalInput")
    ident_d = nc.dram_tensor("ident", [P, P], BF, kind="ExternalInput")
    out_d = nc.dram_tensor("out_part", [GSLOTS, DOUT], F32, kind="ExternalOutput")

    h1s_d = nc.dram_tensor("h1s", [cfg.nshard, H], BF)           # shard (collective in)
    h1f_d = nc.dram_tensor("h1f", [cfg.N, H], BF, addr_space="Shared")  # collective out

    ns = cfg.nshard
    L = 2

    with tile.TileContext(nc) as tc:
        with (
            tc.tile_pool(name="const", bufs=1) as cpool,
            tc.tile_pool(name="g0", bufs=2) as g0pool,
            tc.tile_pool(name="g1", bufs=2) as g1pool,
            tc.tile_pool(name="eb", bufs=2) as ebpool,
            tc.tile_pool(name="pairs", bufs=2 * SBLK + 4) as prpool,
            tc.tile_pool(name="z", bufs=2) as zpool,
            tc.tile_pool(name="lnt", bufs=2) as lnpool,
            tc.tile_pool(name="stat", bufs=4) as stpool,
            tc.tile_pool(name="xt", bufs=4) as xtpool,
            tc.tile_pool(name="small", bufs=4) as smpool,
            tc.tile_pool(name="ps_agg", bufs=2, space="PSUM") as ps_agg,
            tc.tile_pool(name="ps_dense", bufs=2, space="PSUM") as ps_dense,
            tc.tile_pool(name="ps_tr", bufs=2, space="PSUM") as ps_tr,
            tc.tile_pool(name="ps_pool", bufs=1, space="PSUM") as ps_pool,
        ):
            # ---------- resident constants ----------
            idx_t = cpool.tile([P, sched.idx_cols], I16)
            nc.sync.dma_start(out=idx_t[:], in_=idx_d.ap())
            epool_t = cpool.tile([P, cfg.npair * GSLOTS], BF)
            nc.sync.dma_start(out=epool_t[:], in_=epool_d.ap())
            vm_t = []
            for i in range(6):
                t = cpool.tile([P, H], BF, tag=f"vm{i}")
                nc.sync.dma_start(out=t[:], in_=vm_d.ap()[i])
                vm_t.append(t)
            ln_t = []
            for i in range(4):
                t = cpool.tile([P, H], F32, tag=f"ln{i}")
                nc.sync.dma_start(out=t[:], in_=lnm_d.ap()[i])
                ln_t.append(t)
            qv_t = []
            for i in range(4):
                t = cpool.tile([1, H], BF, tag=f"qv{i}")
                nc.sync.dma_start(out=t[:], in_=qv_d.ap()[i:i + 1, :])
                qv_t.append(t)
            rsv_t = []
            for i in range(3):
                t = cpool.tile([1, cfg.npad], BF, tag=f"rsv{i}")
                nc.sync.dma_start(out=t[:], in_=rsv_d.ap()[i:i + 1, :])
                rsv_t.append(t)
            pw_t = cpool.tile([H, DOUT], BF)
            nc.sync.dma_start(out=pw_t[:], in_=pw_d.ap())
            ident_t = cpool.tile([P, P], BF)
            nc.sync.dma_start(out=ident_t[:], in_=ident_d.ap())
            h1T_t = cpool.tile([P, cfg.npad], BF)      # feature-major h1 (own shard)
            h1loc_t = cpool.tile([P, cfg.npair * H], BF)  # node-major h1 (own shard)
            eps_t = cpool.tile([P, 1], F32)
            nc.vector.memset(eps_t[:], EPS)

            pool_psum = ps_pool.tile([GSLOTS, H], F32)

            self_incr = [0]  # round-robin counter for SWDGE queues

            def run_layer(l):
                wn1 = float(sched.wn[l, 1])
                ra = float(sched.wa[l, 0] + sched.wa[l, 2])
                ta = float(sched.wa[l, 1])
                ea = float(sched.wa[l, 2])
                g_rep = ln_t[2 * l]
                b_rep = ln_t[2 * l + 1]
                have_b = sched.have_lnb[l]
                bias_mm = sched.have_bias1 if l == 0 else sched.have_bias2
                table = xb_d.ap() if l == 0 else h1f_d.ap()
                tab_lo = table[0:cfg.half]
                tab_hi = table[cfg.half:cfg.N]

                for sb in range(cfg.nsb):
                    b0, b1 = sb * SBLK, min((sb + 1) * SBLK, cfg.nblk)
                    nt0, nt1 = sched.sb_nt[sb]
                    io0, io1 = sched.sb_idx_off[sb]
                    ec0 = sched.sb_ecol[sb]
                    npr = (b1 - b0) // 2
                    pr0 = b0 // 2

                    g0 = g0pool.tile([P, max(sched.nt0_max, 1) * P], BF, tag="g0")
                    g1 = g1pool.tile([P, max(sched.nt1_max, 1) * P], BF, tag="g1")
                    eb = ebpool.tile([P, sched.et_max * P], BF, tag="eb")
                    # dma_gather dies above 1024 indices/instruction (ucode
                    # index-buffer limit) -> chunk into <=8-tile gathers, and
                    # round-robin the 4 SWDGE queues: each queue runs on its
                    # own Q7 core pair, so desc-gen parallelizes ~4x.
                    GMAX = 8
                    for gbuf, ntn, ion, tabn in ((g0, nt0, io0, tab_lo),
                                                 (g1, nt1, io1, tab_hi)):
                        for t0 in range(0, ntn, GMAX):
                            tn = min(GMAX, ntn - t0)
                            nc.gpsimd.dma_gather(
                                out_ap=gbuf[:, t0 * P:(t0 + tn) * P]
                                .rearrange("p (t c) -> p t c", c=P),
                                in_ap=tabn,
                                idxs_ap=idx_t[:, ion + t0 * (P // 16):
                                              ion + (t0 + tn) * (P // 16)],
                                num_idxs=tn * P, num_idxs_reg=tn * P, elem_size=H,
                                queue_num=self_incr[0] % 4)
                            self_incr[0] += 1
                    nc.sync.dma_start(out=eb[:, :(nt0 + nt1) * P],
                                      in_=est_d.ap()[:, ec0:ec0 + (nt0 + nt1) * P])

                    gp = [None] * npr
                    sp = [None] * npr
                    for b in range(b0, b1):
                        tl = sched.block_tiles[b]
                        ps = ps_agg.tile([P, P], F32, tag="agg")
                        for k, (hf, slot, eslot) in enumerate(tl):
                            gsl = g0 if hf == 0 else g1
                            nc.tensor.matmul(
                                ps[:],
                                lhsT=gsl[:, slot * P:(slot + 1) * P],
                                rhs=eb[:, eslot * P:(eslot + 1) * P],
                                start=(k == 0), stop=(k == len(tl) - 1))
                        prl = (b - b0) // 2
                        side = b % 2
                        if side == 0:
                            gp[prl] = prpool.tile([P, P], BF, tag="gp", name=f"gp_{l}_{b}")
                            sp[prl] = prpool.tile([P, P], BF, tag="sp", name=f"sp_{l}_{b}")
                        nc.vector.tensor_copy(out=gp[prl][:, side * BLK:(side + 1) * BLK],
                                              in_=ps[:, 0:BLK])
                        nc.vector.tensor_copy(out=sp[prl][:, side * BLK:(side + 1) * BLK],
                                              in_=ps[:, BLK:2 * BLK])

                    z = zpool.tile([P, max(npr, 1) * H], F32, tag="z")
                    for prl in range(npr):
                        pr = pr0 + prl
                        if l == 0:
                            hT = xtpool.tile([P, P], BF, tag="xt")
                            nc.sync.dma_start(out=hT[:],
                                              in_=xs_d.ap()[pr * P:(pr + 1) * P, :],
                                              transpose=True)
                            hT_ap = hT[:]
                        else:
                            hT_ap = h1T_t[:, pr * P:(pr + 1) * P]
                        po = ps_dense.tile([P, H], F32, tag="dense")
                        nc.tensor.matmul(po[:], lhsT=gp[prl][:], rhs=vm_t[3 * l + 0][:],
                                         start=True, stop=False)
                        nc.tensor.matmul(po[:], lhsT=hT_ap, rhs=vm_t[3 * l + 1][:],
                                         start=False, stop=False)
                        nc.tensor.matmul(po[:], lhsT=sp[prl][:], rhs=vm_t[3 * l + 2][:],
                                         start=False, stop=not bias_mm)
                        if bias_mm:
                            if l == 0:
                                nc.tensor.matmul(po[:], lhsT=rsv_t[0][:, pr * P:(pr + 1) * P],
                                                 rhs=qv_t[0][:], start=False, stop=False)
                                nc.tensor.matmul(po[:], lhsT=rsv_t[1][:, pr * P:(pr + 1) * P],
                                                 rhs=qv_t[1][:], start=False, stop=False)
                                nc.tensor.matmul(po[:], lhsT=rsv_t[2][:, pr * P:(pr + 1) * P],
                                                 rhs=qv_t[2][:], start=False, stop=True)
                            else:
                                nc.tensor.matmul(po[:], lhsT=rsv_t[2][:, pr * P:(pr + 1) * P],
                                                 rhs=qv_t[3][:], start=False, stop=True)
                        nc.vector.tensor_copy(out=z[:, prl * H:(prl + 1) * H], in_=po[:])

                    # ---- fused LayerNorm-mix + activation-mix on [P, npr*H] ----
                    F = npr * H
                    z3 = z[:, :F].rearrange("p (g c) -> p g c", c=H)
                    mu = stpool.tile([P, max(npr, 1)], F32, tag="mu")
                    nc.vector.tensor_reduce(out=mu[:, :npr], in_=z3,
                                            axis=mybir.AxisListType.X, op=mybir.AluOpType.add)
                    nc.vector.tensor_scalar_mul(mu[:, :npr], mu[:, :npr], 1.0 / H)
                    zc = lnpool.tile([P, max(npr, 1) * H], F32, tag="zc")
                    nc.vector.tensor_tensor(out=zc[:, :F].rearrange("p (g c) -> p g c", c=H),
                                            in0=z3,
                                            in1=mu[:, :npr].to_broadcast([P, npr, H]),
                                            op=mybir.AluOpType.subtract)
                    sq = lnpool.tile([P, max(npr, 1) * H], F32, tag="sq")
                    nc.scalar.square(out=sq[:, :F], in_=zc[:, :F])
                    var = stpool.tile([P, max(npr, 1)], F32, tag="var")
                    nc.vector.tensor_reduce(out=var[:, :npr],
                                            in_=sq[:, :F].rearrange("p (g c) -> p g c", c=H),
                                            axis=mybir.AxisListType.X, op=mybir.AluOpType.add)
                    sd = stpool.tile([P, max(npr, 1)], F32, tag="sd")
                    nc.scalar.activation(out=sd[:, :npr], in_=var[:, :npr],
                                         func=mybir.ActivationFunctionType.Sqrt,
                                         bias=eps_t[:], scale=1.0 / H)
                    rsl = stpool.tile([P, max(npr, 1)], F32, tag="rsl")
                    nc.vector.reciprocal(out=rsl[:, :npr], in_=sd[:, :npr])
                    u = lnpool.tile([P, max(npr, 1) * H], F32, tag="u")
                    nc.vector.tensor_tensor(out=u[:, :F].rearrange("p (g c) -> p g c", c=H),
                                            in0=zc[:, :F].rearrange("p (g c) -> p g c", c=H),
                                            in1=rsl[:, :npr].to_broadcast([P, npr, H]),
                                            op=mybir.AluOpType.mult)
                    g_bc = dataclasses.replace(g_rep[:], ap=[g_rep[:].ap[0], [0, npr],
                                                             g_rep[:].ap[1]])
                    v = lnpool.tile([P, max(npr, 1) * H], F32, tag="v")
                    nc.vector.tensor_tensor(out=v[:, :F].rearrange("p (g c) -> p g c", c=H),
                                            in0=u[:, :F].rearrange("p (g c) -> p g c", c=H),
                                            in1=g_bc, op=mybir.AluOpType.mult)
                    w = zc  # reuse
                    nc.vector.tensor_scalar_mul(w[:, :F], z[:, :F], wn1)
                    hpre = u  # reuse
                    nc.vector.tensor_tensor(out=hpre[:, :F], in0=v[:, :F], in1=w[:, :F],
                                            op=mybir.AluOpType.add)
                    if have_b:
                        b_bc = dataclasses.replace(b_rep[:], ap=[b_rep[:].ap[0], [0, npr],
                                                                 b_rep[:].ap[1]])
                        nc.vector.tensor_tensor(
                            out=hpre[:, :F].rearrange("p (g c) -> p g c", c=H),
                            in0=hpre[:, :F].rearrange("p (g c) -> p g c", c=H),
                            in1=b_bc, op=mybir.AluOpType.add)
                    # activation mix: (wa0+wa2)*relu(x) + wa1*tanh(x) + wa2*exp(min(x,0)) - wa2
                    r_t = v  # reuse
                    nc.scalar.activation(out=r_t[:, :F], in_=hpre[:, :F],
                                         func=mybir.ActivationFunctionType.Relu, scale=ra)
                    th_t = sq  # reuse
                    nc.scalar.activation(out=th_t[:, :F], in_=hpre[:, :F],
                                         func=mybir.ActivationFunctionType.Tanh)
                    m_t = w  # reuse (zc)
                    nc.vector.tensor_scalar_min(m_t[:, :F], hpre[:, :F], 0.0)
                    e_t = z  # reuse z
                    nc.scalar.activation(out=e_t[:, :F], in_=m_t[:, :F],
                                         func=mybir.ActivationFunctionType.Exp)
                    nc.vector.tensor_scalar_mul(th_t[:, :F], th_t[:, :F], ta)
                    nc.vector.tensor_scalar(out=e_t[:, :F], in0=e_t[:, :F],
                                            scalar1=ea, scalar2=-ea,
                                            op0=mybir.AluOpType.mult,
                                            op1=mybir.AluOpType.add)
                    nc.vector.tensor_tensor(out=r_t[:, :F], in0=r_t[:, :F],
                                            in1=th_t[:, :F], op=mybir.AluOpType.add)
                    if l == 0:
                        hdst = h1loc_t[:, pr0 * H:pr0 * H + F]
                    else:
                        h2sb = lnpool.tile([P, max(npr, 1) * H], BF, tag="h2")
                        hdst = h2sb[:, :F]
                    nc.vector.tensor_tensor(out=hdst, in0=r_t[:, :F], in1=e_t[:, :F],
                                            op=mybir.AluOpType.add)

                    if l == 0:
                        for prl in range(npr):
                            pr = pr0 + prl
                            rows = min(P, ns - pr * P)
                            if rows > 0:
                                nc.sync.dma_start(
                                    out=h1s_d.ap()[pr * P:pr * P + rows, :],
                                    in_=h1loc_t[0:rows, pr * H:(pr + 1) * H])
                            pt = ps_tr.tile([P, P], BF, tag="tr")
                            nc.tensor.transpose(out=pt[:],
                                                in_=h1loc_t[:, pr * H:(pr + 1) * H],
                                                identity=ident_t[:])
                            nc.vector.tensor_copy(out=h1T_t[:, pr * P:(pr + 1) * P],
                                                  in_=pt[:])
                    else:
                        skip = lnpool.tile([P, max(npr, 1) * H], BF, tag="skip")
                        nc.vector.tensor_tensor(out=skip[:, :F],
                                                in0=h1loc_t[:, pr0 * H:pr0 * H + F],
                                                in1=hdst, op=mybir.AluOpType.add)
                        for prl in range(npr):
                            pr = pr0 + prl
                            nc.tensor.matmul(
                                pool_psum[:],
                                lhsT=epool_t[:, pr * GSLOTS:(pr + 1) * GSLOTS],
                                rhs=skip[:, prl * H:(prl + 1) * H],
                                start=(pr == 0), stop=(pr == cfg.npair - 1))

            run_layer(0)
            nc.gpsimd.collective_compute(
                "AllGather", mybir.AluOpType.bypass,
                replica_groups=[list(range(cfg.cores))],
                ins=[h1s_d.ap()], outs=[h1f_d.ap()])
            run_layer(1)

            # ---------- readout: pooled @ post_w ----------
            poolc = smpool.tile([GSLOTS, H], BF, tag="poolc")
            nc.vector.tensor_copy(out=poolc[:], in_=pool_psum[:])
            pt = ps_tr.tile([P, GSLOTS], BF, tag="tr")
            nc.tensor.transpose(out=pt[:], in_=poolc[:], identity=ident_t[:])
            ptc = smpool.tile([P, GSLOTS], BF, tag="ptc")
            nc.vector.tensor_copy(out=ptc[:], in_=pt[:])
            ops = ps_dense.tile([GSLOTS, DOUT], F32, tag="dense")
            nc.tensor.matmul(ops[:], lhsT=ptc[:], rhs=pw_t[:], start=True, stop=True)
            outc = smpool.tile([GSLOTS, DOUT], F32, tag="outc")
            nc.vector.tensor_copy(out=outc[:], in_=ops[:])
            nc.sync.dma_start(out=out_d.ap(), in_=outc[:])

    nc.compile()
    return nc


def _kernel_impl(inputs: dict, cfg: Cfg = None, trace: bool = False):
    if cfg is None:
        cfg = Cfg(N=50000, E=640000, G=500, cores=8, half=32768)
    sched, data, combine = host_prep(inputs, cfg)
    nc = build_program(cfg, sched)
    in_maps = [data[c] for c in range(cfg.cores)]
    res = run_bass_kernel_spmd(nc, in_maps, core_ids=list(range(cfg.cores)),
                               trace=trace)
    out = np.zeros((cfg.G, DOUT), np.float64)
    for c in range(cfg.cores):
        part = np.asarray(res.results[c]["out_part"], np.float64)
        lo = combine["g_lo"][c]
        hi = min(lo + GSLOTS, cfg.G)
        out[lo:hi] += part[:hi - lo]
    out += combine["post_b"]
    return out.astype(np.float32), res


def kernel(**inputs) -> np.ndarray:
    out, _ = _kernel_impl(inputs)
    return out


# revision 18
# speedup vs baseline: 1.7487x; 1.0763x over previous
"""Trainium2 Bass kernel for nn_MicroCoupledSuperNet (GNN message passing supernet).

Strategy (8-core SPMD, dst-node sharding):
  - Each core owns a contiguous range of destination nodes and all edges into them.
  - Per layer, both GCN (sym-normalized, self-loops) and SAGE-mean aggregations are
    computed with ONE matmul per 128-edge tile: gathered-source-rows^T @ E, where
    E in bf16 carries the per-edge weights (gcn_norm | 1/deg) into a combined
    [64 gcn cols | 64 sage cols] block of 64 destination nodes, accumulated in PSUM.
  - Source rows are fetched with dma_gather (int16 indices -> table split in two halves).
  - pre-MLP is deferred through the aggregation (A(xW) = (Ax)W), so layer 1 gathers
    straight from the x table; the dense stage fuses conv-mix into 3 matmuls per
    128-node block-pair, followed by a fused LayerNorm-mix + activation-mix chain.
  - h1 is exchanged between layers with an AllGather collective.
  - Sum-pool readout is a 0/1 matmul into per-core graph slots; host merges windows
    and adds post_b.
"""

import sys
import math
import dataclasses

import numpy as np

for _p in ("/opt/trn_rl_repo",):
    if _p not in sys.path:
        sys.path.insert(0, _p)

import ml_dtypes  # noqa: E402

BF16 = ml_dtypes.bfloat16

from concourse import bass, bacc, mybir, tile  # noqa: E402
from concourse.bass_utils import run_bass_kernel_spmd  # noqa: E402

P = 128          # SBUF partitions / edge-tile rows
BLK = 64         # destination nodes per aggregation block
H = 128          # hidden dim (== D_IN)
DOUT = 64
SBLK = 8         # aggregation blocks per superblock (scheduling unit)
GSLOTS = 128     # per-core graph slots for pooling
EPS = 1e-5
F32 = mybir.dt.float32
BF = mybir.dt.bfloat16
I16 = mybir.dt.int16


@dataclasses.dataclass
class Cfg:
    N: int
    E: int
    G: int
    cores: int
    half: int           # gather table split point (int16 index limit)
    sim_pad_zero: bool = False   # sim asserts num_idxs_reg == count(>=0)
    nshard: int = 0
    nblk: int = 0
    npair: int = 0
    npad: int = 0
    nsb: int = 0

    def __post_init__(self):
        assert self.N % self.cores == 0
        self.nshard = self.N // self.cores
        self.nblk = math.ceil(self.nshard / BLK)
        if self.nblk % 2:
            self.nblk += 1  # keep whole pairs
        self.npair = self.nblk // 2
        self.npad = self.nblk * BLK
        self.nsb = math.ceil(self.nblk / SBLK)


def _softmax(v):
    v = np.asarray(v, np.float64)
    e = np.exp(v - v.max())
    return e / e.sum()


@dataclasses.dataclass
class Sched:
    """Static (cross-core-uniform) schedule + scalar constants."""
    T: np.ndarray            # [nblk, 2] tiles per (block, half)
    Tc: np.ndarray           # [nblk, 2] gathered idx count per bucket (x16)
    b_idx_off: list          # per block: idx col offset (h0 tiles then h1)
    b_ecol: list             # per block: E-stream col offset
    idx_cols: int
    ecols: int
    etb_max: int             # max tiles per block (both halves)
    # scalar constants per layer
    wc: np.ndarray           # [L,2]
    wn: np.ndarray           # [L,2]
    wa: np.ndarray           # [L,3]
    have_bias1: bool
    have_bias2: bool
    have_lnb: list           # per layer: B row nonzero
    shard_rows: int          # real rows per shard (nshard)


def _build_schedule(cfg: Cfg, counts: np.ndarray) -> tuple:
    """counts: [cores, nblk, 2] edge counts. Returns tile schedule uniform across cores.
    Streams are block-major: block b's h0 tiles then h1 tiles, contiguous."""
    mx = counts.max(axis=0)
    Tc = (np.ceil(mx / 16) * 16).astype(np.int64)          # gathered idxs (x16)
    T = np.ceil(mx / P).astype(np.int64)                   # matmul tiles
    b_idx_off, b_ecol = [], []
    idx_off = 0
    ecol = 0
    for b in range(cfg.nblk):
        b_idx_off.append(idx_off)
        b_ecol.append(ecol)
        idx_off += int(Tc[b, 0] + Tc[b, 1]) // 16
        ecol += int(T[b, 0] + T[b, 1]) * P
    etb_max = int((T[:, 0] + T[:, 1]).max())
    return T, Tc, b_idx_off, b_ecol, idx_off, ecol, etb_max


def host_prep(inputs: dict, cfg: Cfg):
    """Numpy preprocessing: edge bucketing/tiling, E-matrix stream, index stream,
    combined weight matrices. Returns (sched, per-core in_maps data, combine info)."""
    x = np.asarray(inputs["x"], np.float32)
    ei = np.asarray(inputs["edge_index"])
    batch = np.asarray(inputs["batch"]).astype(np.int64)
    src = ei[0].astype(np.int64)
    dst = ei[1].astype(np.int64)
    N, E, G_N, C = cfg.N, cfg.E, cfg.G, cfg.cores
    ns = cfg.nshard

    deg_sl = np.bincount(dst, minlength=N).astype(np.float64) + 1.0  # with self loop
    dinv = 1.0 / np.sqrt(deg_sl)
    degn = np.maximum(np.bincount(dst, minlength=N), 1).astype(np.float64)

    # ---- per-core edge lists (with self-loop pseudo-edges) ----
    per_core = []
    counts = np.zeros((C, cfg.nblk, 2), np.int64)
    for c in range(C):
        lo, hi = c * ns, (c + 1) * ns
        m = (dst >= lo) & (dst < hi)
        es, ed = src[m], dst[m]
        dd = np.arange(lo, hi, dtype=np.int64)
        asrc = np.concatenate([es, dd])
        adst = np.concatenate([ed, dd])
        wg = np.concatenate([dinv[es] * dinv[ed], dinv[dd] ** 2])
        ws = np.concatenate([1.0 / degn[ed], np.zeros(ns)])
        dloc = adst - lo
        blk = dloc // BLK
        din = dloc % BLK
        hf = (asrc >= cfg.half).astype(np.int64)
        order = np.lexsort((hf, blk))
        asrc, wg, ws, blk, din, hf = (a[order] for a in (asrc, wg, ws, blk, din, hf))
        for b in range(cfg.nblk):
            mb = blk == b
            counts[c, b, 0] = int((mb & (hf == 0)).sum())
            counts[c, b, 1] = int((mb & (hf == 1)).sum())
        per_core.append((asrc, wg, ws, blk, din, hf))

    T, Tc, b_idx_off, b_ecol, idx_cols, ecols, etb_max = _build_schedule(cfg, counts)

    # ---- pack per-core index + E streams ----
    data = []
    for c in range(C):
        asrc, wg, ws, blk, din, hf = per_core[c]
        # slot assignment: edges of (b, h) fill first counts[c,b,h] slots of its tiles
        idx_parts = []   # in gather-stream order (sb, half, block, tile)
        n_tiles_total = int(T.sum())
        Efull = np.zeros((n_tiles_total, P, P), np.float32)
        # global tile index per (b, h): block-major, h0 then h1 within a block
        tile_base = {}
        idx_base = {}
        tix = 0
        cix = 0
        for b in range(cfg.nblk):
            for hh in (0, 1):
                tile_base[(b, hh)] = tix
                idx_base[(b, hh)] = cix
                tix += int(T[b, hh])
                cix += int(Tc[b, hh])
        assert tix == n_tiles_total
        idx_total = cix
        # scatter edges into tiles
        key = blk * 2 + hf
        order = np.argsort(key, kind="stable")
        asrc, wg, ws, blk, din, hf = (a[order] for a in (asrc, wg, ws, blk, din, hf))
        # position within (b, h) bucket
        pos = np.zeros(len(asrc), np.int64)
        start = 0
        for b in range(cfg.nblk):
            for hh in (0, 1):
                nbh = counts[c, b, hh]
                pos[start:start + nbh] = np.arange(nbh)
                start += nbh
        tno = np.array([tile_base[(int(b), int(h))] for b, h in zip(blk, hf)]) + pos // P
        prow = pos % P
        idxval = np.where(hf == 0, asrc, asrc - cfg.half)
        Efull[tno, prow, din] = wg
        Efull[tno, prow, BLK + din] = ws
        # E stream partition-major [P, n_tiles*P]
        est = np.ascontiguousarray(
            Efull.transpose(1, 0, 2).reshape(P, n_tiles_total * P)).astype(BF16)
        # idx stream: per-bucket Tc-sized ranges (gathers run at 16-idx
        # granularity; pads use index 0 and zero E weight)
        ipos = np.array([idx_base[(int(b), int(h))] for b, h in zip(blk, hf)]) + pos
        flat = np.zeros(idx_total, np.int64)
        flat[ipos] = idxval
        wrapped = flat.reshape(-1, 16).T  # [16, total/16]
        idx16 = np.tile(wrapped, (8, 1)).astype(np.int16)  # [128, cols]
        assert idx16.shape[1] == idx_cols
        data.append({"est": est, "idx": idx16})

    # ---- pooling ----
    g_lo = []
    for c in range(C):
        lo = int(batch[c * ns])
        hi = int(batch[(c + 1) * ns - 1])
        span = hi - lo + 1
        assert span <= GSLOTS, f"graph span {span} exceeds {GSLOTS}"
        g_lo.append(lo)
        ep = np.zeros((cfg.npad, GSLOTS), np.float32)
        rows = np.arange(ns)
        ep[rows, batch[c * ns:(c + 1) * ns] - lo] = 1.0
        epm = np.ascontiguousarray(
            ep.reshape(cfg.npair, P, GSLOTS).transpose(1, 0, 2)
            .reshape(P, cfg.npair * GSLOTS)).astype(BF16)
        data[c]["epool"] = epm

    # ---- weights / constants ----
    pre_w = np.asarray(inputs["pre_w"], np.float64)
    pre_b = np.asarray(inputs["pre_b"], np.float64)
    post_w = np.asarray(inputs["post_w"], np.float64)
    post_b = np.asarray(inputs["post_b"], np.float64)
    gcn_w = np.asarray(inputs["gcn_w"], np.float64)
    gcn_b = np.asarray(inputs["gcn_b"], np.float64)
    sage_ws = np.asarray(inputs["sage_ws"], np.float64)
    sage_wn = np.asarray(inputs["sage_wn"], np.float64)
    ln_g = np.asarray(inputs["ln_g"], np.float64)
    ln_b = np.asarray(inputs["ln_b"], np.float64)
    a_conv = np.asarray(inputs["a_conv"], np.float64)
    a_norm = np.asarray(inputs["a_norm"], np.float64)
    a_act = np.asarray(inputs["a_act"], np.float64)

    wc = np.stack([_softmax(a_conv[l]) for l in range(2)])
    wn = np.stack([_softmax(a_norm[l]) for l in range(2)])
    wa = np.stack([_softmax(a_act[l]) for l in range(2)])

    Vg1 = pre_w @ (wc[0, 0] * gcn_w[0])
    VI1 = pre_w @ (wc[0, 1] * sage_ws[0])
    Vs1 = pre_w @ (wc[0, 1] * sage_wn[0])
    Vg2 = wc[1, 0] * gcn_w[1]
    VI2 = wc[1, 1] * sage_ws[1]
    Vs2 = wc[1, 1] * sage_wn[1]
    vm = np.stack([Vg1, VI1, Vs1, Vg2, VI2, Vs2]).astype(BF16)

    qg = wc[0, 0] * (pre_b @ gcn_w[0])
    qs = wc[0, 1] * (pre_b @ sage_wn[0])
    qc = wc[0, 0] * gcn_b[0] + wc[0, 1] * (pre_b @ sage_ws[0])
    bc2 = wc[1, 0] * gcn_b[1]
    qv = np.stack([qg, qs, qc, bc2]).astype(BF16)
    have_bias1 = bool(np.abs(qv[:3]).max() > 0)
    have_bias2 = bool(np.abs(bc2).max() > 0)

    # rs vectors (per-core, padded)
    rs_gcn_full = np.zeros(N)
    np.add.at(rs_gcn_full, dst, dinv[src])
    rs_gcn_full = dinv * rs_gcn_full + dinv ** 2
    rs_sage_full = (np.bincount(dst, minlength=N) > 0).astype(np.float64)
    for c in range(C):
        r = np.zeros((3, cfg.npad), np.float32)
        r[0, :ns] = rs_gcn_full[c * ns:(c + 1) * ns]
        r[1, :ns] = rs_sage_full[c * ns:(c + 1) * ns]
        r[2, :] = 1.0
        data[c]["rsv"] = r.astype(BF16)

    G1 = wn[0, 0] * ln_g[0]
    B1 = wn[0, 0] * ln_b[0]
    G2 = wn[1, 0] * ln_g[1]
    B2 = wn[1, 0] * ln_b[1]
    lnm = np.stack([np.tile(G1, (P, 1)), np.tile(B1, (P, 1)),
                    np.tile(G2, (P, 1)), np.tile(B2, (P, 1))]).astype(np.float32)
    have_lnb = [bool(np.abs(B1).max() > 0), bool(np.abs(B2).max() > 0)]

    xb = x.astype(BF16)  # global gather table
    for c in range(C):
        xs = np.zeros((cfg.npad, H), np.float32)
        xs[:ns] = x[c * ns:(c + 1) * ns]
        data[c]["xst"] = np.ascontiguousarray(xs.T).astype(BF16)
        data[c]["xb"] = xb
        data[c]["vm"] = vm
        data[c]["qv"] = qv
        data[c]["lnm"] = lnm
        data[c]["pw"] = post_w.astype(BF16)
        data[c]["ident"] = np.eye(P, dtype=np.float32).astype(BF16)

    sched = Sched(T=T, Tc=Tc, b_idx_off=b_idx_off, b_ecol=b_ecol,
                  idx_cols=idx_cols, ecols=ecols, etb_max=etb_max,
                  wc=wc, wn=wn, wa=wa,
                  have_bias1=have_bias1, have_bias2=have_bias2,
                  have_lnb=have_lnb, shard_rows=ns)
    combine = {"g_lo": g_lo, "post_b": post_b}
    return sched, data, combine


def build_program(cfg: Cfg, sched: Sched):
    nc = bacc.Bacc("TRN2", target_bir_lowering=False, debug=False,
                   enable_asserts=False, num_devices=cfg.cores,
                   num_swdge_queues=4)

    xb_d = nc.dram_tensor("xb", [cfg.N, H], BF, kind="ExternalInput")
    xst_d = nc.dram_tensor("xst", [H, cfg.npad], BF, kind="ExternalInput")
    idx_d = nc.dram_tensor("idx", [P, sched.idx_cols], I16, kind="ExternalInput")
    est_d = nc.dram_tensor("est", [P, sched.ecols], BF, kind="ExternalInput")
    epool_d = nc.dram_tensor("epool", [P, cfg.npair * GSLOTS], BF, kind="ExternalInput")
    vm_d = nc.dram_tensor("vm", [6, P, H], BF, kind="ExternalInput")
    qv_d = nc.dram_tensor("qv", [4, H], BF, kind="ExternalInput")
    rsv_d = nc.dram_tensor("rsv", [3, cfg.npad], BF, kind="ExternalInput")
    lnm_d = nc.dram_tensor("lnm", [4, P, H], F32, kind="ExternalInput")
    pw_d = nc.dram_tensor("pw", [H, DOUT], BF, kind="ExternalInput")
    ident_d = nc.dram_tensor("ident", [P, P], BF, kind="ExternalInput")
    out_d = nc.dram_tensor("out_part", [GSLOTS, DOUT], F32, kind="ExternalOutput")

    h1s_d = nc.dram_tensor("h1s", [cfg.nshard, H], BF)           # shard (collective in)
    h1f_d = nc.dram_tensor("h1f", [cfg.N, H], BF, addr_space="Shared")  # collective out

    ns = cfg.nshard
    L = 2

    with tile.TileContext(nc) as tc:
        with (
            tc.tile_pool(name="const", bufs=1) as cpool,
            tc.tile_pool(name="gb", bufs=4) as gbpool,
            tc.tile_pool(name="eb", bufs=4) as ebpool,
            tc.tile_pool(name="pairs", bufs=2 * SBLK + 4) as prpool,
            tc.tile_pool(name="z", bufs=2) as zpool,
            tc.tile_pool(name="lnt", bufs=2) as lnpool,
            tc.tile_pool(name="stat", bufs=4) as stpool,
            tc.tile_pool(name="xt", bufs=4) as xtpool,
            tc.tile_pool(name="small", bufs=4) as smpool,
            tc.tile_pool(name="ps_agg", bufs=2, space="PSUM") as ps_agg,
            tc.tile_pool(name="ps_dense", bufs=2, space="PSUM") as ps_dense,
            tc.tile_pool(name="ps_tr", bufs=2, space="PSUM") as ps_tr,
            tc.tile_pool(name="ps_pool", bufs=1, space="PSUM") as ps_pool,
        ):
            # ---------- resident constants ----------
            idx_t = cpool.tile([P, sched.idx_cols], I16)
            nc.sync.dma_start(out=idx_t[:], in_=idx_d.ap())
            epool_t = cpool.tile([P, cfg.npair * GSLOTS], BF)
            nc.sync.dma_start(out=epool_t[:], in_=epool_d.ap())
            vm_t = []
            for i in range(6):
                t = cpool.tile([P, H], BF, tag=f"vm{i}")
                nc.sync.dma_start(out=t[:], in_=vm_d.ap()[i])
                vm_t.append(t)
            ln_t = []
            for i in range(4):
                t = cpool.tile([P, H], F32, tag=f"ln{i}")
                nc.sync.dma_start(out=t[:], in_=lnm_d.ap()[i])
                ln_t.append(t)
            qv_t = []
            for i in range(4):
                t = cpool.tile([1, H], BF, tag=f"qv{i}")
                nc.sync.dma_start(out=t[:], in_=qv_d.ap()[i:i + 1, :])
                qv_t.append(t)
            rsv_t = []
            for i in range(3):
                t = cpool.tile([1, cfg.npad], BF, tag=f"rsv{i}")
                nc.sync.dma_start(out=t[:], in_=rsv_d.ap()[i:i + 1, :])
                rsv_t.append(t)
            pw_t = cpool.tile([H, DOUT], BF)
            nc.sync.dma_start(out=pw_t[:], in_=pw_d.ap())
            ident_t = cpool.tile([P, P], BF)
            nc.sync.dma_start(out=ident_t[:], in_=ident_d.ap())
            xst_t = cpool.tile([P, cfg.npad], BF)      # feature-major x (own shard)
            nc.sync.dma_start(out=xst_t[:], in_=xst_d.ap())
            h1T_t = cpool.tile([P, cfg.npad], BF)      # feature-major h1 (own shard)
            h1loc_t = cpool.tile([P, cfg.npair * H], BF)  # node-major h1 (own shard)
            eps_t = cpool.tile([P, 1], F32)
            nc.vector.memset(eps_t[:], EPS)

            pool_psum = ps_pool.tile([GSLOTS, H], F32)

            self_incr = [0]  # round-robin counter for SWDGE queues

            def run_layer(l):
                wn1 = float(sched.wn[l, 1])
                ra = float(sched.wa[l, 0] + sched.wa[l, 2])
                ta = float(sched.wa[l, 1])
                ea = float(sched.wa[l, 2])
                g_rep = ln_t[2 * l]
                b_rep = ln_t[2 * l + 1]
                have_b = sched.have_lnb[l]
                bias_mm = sched.have_bias1 if l == 0 else sched.have_bias2
                table = xb_d.ap() if l == 0 else h1f_d.ap()
                tab_lo = table[0:cfg.half]
                tab_hi = table[cfg.half:cfg.N]

                for sb in range(cfg.nsb):
                    b0, b1 = sb * SBLK, min((sb + 1) * SBLK, cfg.nblk)
                    npr = (b1 - b0) // 2
                    pr0 = b0 // 2

                    gp = [None] * npr
                    sp = [None] * npr
                    for b in range(b0, b1):
                        nt0 = int(sched.T[b, 0])
                        nt1 = int(sched.T[b, 1])
                        ntb = nt0 + nt1
                        iob = sched.b_idx_off[b]
                        ecb = sched.b_ecol[b]
                        gb = gbpool.tile([P, sched.etb_max * P], BF, tag="gb",
                                         name=f"gb_{l}_{b}")
                        eb = ebpool.tile([P, sched.etb_max * P], BF, tag="ebb",
                                         name=f"eb_{l}_{b}")
                        if l == 0 and b < 4:
                            # first touch of each rotating pool slot: zero it
                            # so rows skipped by the trailing-negative trim
                            # stay finite
                            nc.vector.memset(gb[:], 0)
                        # dma_gather dies above 1024 indices/instruction
                        # (ucode index-buffer limit) -> one gather per
                        # (block, half) bucket, so each bucket's tail padding
                        # is trailing -1s the ucode trims without fetching.
                        # Round-robin the 4 SWDGE queues: each queue runs on
                        # its own Q7 core pair -> ~4x desc-gen parallelism.
                        nc0 = int(sched.Tc[b, 0])
                        nc1 = int(sched.Tc[b, 1])
                        for hh, t0, tn, cn, co in ((0, 0, nt0, nc0, 0),
                                                   (1, nt0, nt1, nc1, nc0)):
                            if cn == 0:
                                continue
                            assert cn <= 1024, "bucket exceeds gather limit"
                            tabn = tab_lo if hh == 0 else tab_hi
                            nc.gpsimd.dma_gather(
                                out_ap=gb[:, t0 * P:(t0 + tn) * P]
                                .rearrange("p (t c) -> p t c", c=P),
                                in_ap=tabn,
                                idxs_ap=idx_t[:, iob + co // 16:
                                              iob + (co + cn) // 16],
                                num_idxs=cn, num_idxs_reg=cn, elem_size=H,
                                queue_num=self_incr[0] % 4)
                            self_incr[0] += 1
                        nc.sync.dma_start(out=eb[:, :ntb * P],
                                          in_=est_d.ap()[:, ecb:ecb + ntb * P])

                        ps = ps_agg.tile([P, P], F32, tag="agg")
                        for k in range(ntb):
                            nc.tensor.matmul(
                                ps[:],
                                lhsT=gb[:, k * P:(k + 1) * P],
                                rhs=eb[:, k * P:(k + 1) * P],
                                start=(k == 0), stop=(k == ntb - 1))
                        prl = (b - b0) // 2
                        side = b % 2
                        if side == 0:
                            gp[prl] = prpool.tile([P, P], BF, tag="gp", name=f"gp_{l}_{b}")
                            sp[prl] = prpool.tile([P, P], BF, tag="sp", name=f"sp_{l}_{b}")
                        nc.vector.tensor_copy(out=gp[prl][:, side * BLK:(side + 1) * BLK],
                                              in_=ps[:, 0:BLK])
                        nc.vector.tensor_copy(out=sp[prl][:, side * BLK:(side + 1) * BLK],
                                              in_=ps[:, BLK:2 * BLK])

                    z = zpool.tile([P, max(npr, 1) * H], F32, tag="z")
                    for prl in range(npr):
                        pr = pr0 + prl
                        hsrc = xst_t if l == 0 else h1T_t
                        hT_ap = hsrc[:, pr * P:(pr + 1) * P]
                        po = ps_dense.tile([P, H], F32, tag="dense")
                        nc.tensor.matmul(po[:], lhsT=gp[prl][:], rhs=vm_t[3 * l + 0][:],
                                         start=True, stop=False)
                        nc.tensor.matmul(po[:], lhsT=hT_ap, rhs=vm_t[3 * l + 1][:],
                                         start=False, stop=False)
                        nc.tensor.matmul(po[:], lhsT=sp[prl][:], rhs=vm_t[3 * l + 2][:],
                                         start=False, stop=not bias_mm)
                        if bias_mm:
                            if l == 0:
                                nc.tensor.matmul(po[:], lhsT=rsv_t[0][:, pr * P:(pr + 1) * P],
                                                 rhs=qv_t[0][:], start=False, stop=False)
                                nc.tensor.matmul(po[:], lhsT=rsv_t[1][:, pr * P:(pr + 1) * P],
                                                 rhs=qv_t[1][:], start=False, stop=False)
                                nc.tensor.matmul(po[:], lhsT=rsv_t[2][:, pr * P:(pr + 1) * P],
                                                 rhs=qv_t[2][:], start=False, stop=True)
                            else:
                                nc.tensor.matmul(po[:], lhsT=rsv_t[2][:, pr * P:(pr + 1) * P],
                                                 rhs=qv_t[3][:], start=False, stop=True)
                        nc.vector.tensor_copy(out=z[:, prl * H:(prl + 1) * H], in_=po[:])

                    # ---- fused LayerNorm-mix + activation-mix on [P, npr*H] ----
                    F = npr * H
                    z3 = z[:, :F].rearrange("p (g c) -> p g c", c=H)
                    mu = stpool.tile([P, max(npr, 1)], F32, tag="mu")
                    nc.vector.tensor_reduce(out=mu[:, :npr], in_=z3,
                                            axis=mybir.AxisListType.X, op=mybir.AluOpType.add)
                    nc.vector.tensor_scalar_mul(mu[:, :npr], mu[:, :npr], 1.0 / H)
                    zc = lnpool.tile([P, max(npr, 1) * H], F32, tag="zc")
                    nc.vector.tensor_tensor(out=zc[:, :F].rearrange("p (g c) -> p g c", c=H),
                                            in0=z3,
                                            in1=mu[:, :npr].to_broadcast([P, npr, H]),
                                            op=mybir.AluOpType.subtract)
                    sq = lnpool.tile([P, max(npr, 1) * H], F32, tag="sq")
                    nc.scalar.square(out=sq[:, :F], in_=zc[:, :F])
                    var = stpool.tile([P, max(npr, 1)], F32, tag="var")
                    nc.vector.tensor_reduce(out=var[:, :npr],
                                            in_=sq[:, :F].rearrange("p (g c) -> p g c", c=H),
                                            axis=mybir.AxisListType.X, op=mybir.AluOpType.add)
                    sd = stpool.tile([P, max(npr, 1)], F32, tag="sd")
                    nc.scalar.activation(out=sd[:, :npr], in_=var[:, :npr],
                                         func=mybir.ActivationFunctionType.Sqrt,
                                         bias=eps_t[:], scale=1.0 / H)
                    rsl = stpool.tile([P, max(npr, 1)], F32, tag="rsl")
                    nc.vector.reciprocal(out=rsl[:, :npr], in_=sd[:, :npr])
                    u = lnpool.tile([P, max(npr, 1) * H], F32, tag="u")
                    nc.vector.tensor_tensor(out=u[:, :F].rearrange("p (g c) -> p g c", c=H),
                                            in0=zc[:, :F].rearrange("p (g c) -> p g c", c=H),
                                            in1=rsl[:, :npr].to_broadcast([P, npr, H]),
                                            op=mybir.AluOpType.mult)
                    g_bc = dataclasses.replace(g_rep[:], ap=[g_rep[:].ap[0], [0, npr],
                                                             g_rep[:].ap[1]])
                    v = u  # in-place scale by the G row
                    nc.vector.tensor_tensor(out=v[:, :F].rearrange("p (g c) -> p g c", c=H),
                                            in0=u[:, :F].rearrange("p (g c) -> p g c", c=H),
                                            in1=g_bc, op=mybir.AluOpType.mult)
                    w = zc  # reuse
                    nc.vector.tensor_scalar_mul(w[:, :F], z[:, :F], wn1)
                    hpre = u  # reuse
                    nc.vector.tensor_tensor(out=hpre[:, :F], in0=v[:, :F], in1=w[:, :F],
                                            op=mybir.AluOpType.add)
                    if have_b:
                        b_bc = dataclasses.replace(b_rep[:], ap=[b_rep[:].ap[0], [0, npr],
                                                                 b_rep[:].ap[1]])
                        nc.vector.tensor_tensor(
                            out=hpre[:, :F].rearrange("p (g c) -> p g c", c=H),
                            in0=hpre[:, :F].rearrange("p (g c) -> p g c", c=H),
                            in1=b_bc, op=mybir.AluOpType.add)
                    # activation mix: (wa0+wa2)*relu(x) + wa1*tanh(x) + wa2*exp(min(x,0)) - wa2
                    th_t = sq  # reuse
                    nc.scalar.activation(out=th_t[:, :F], in_=hpre[:, :F],
                                         func=mybir.ActivationFunctionType.Tanh)
                    m_t = w  # reuse (zc)
                    nc.vector.tensor_scalar_min(m_t[:, :F], hpre[:, :F], 0.0)
                    e_t = z  # reuse z
                    nc.scalar.activation(out=e_t[:, :F], in_=m_t[:, :F],
                                         func=mybir.ActivationFunctionType.Exp)
                    r_t = hpre  # in-place: relu is the last reader of hpre
                    nc.scalar.activation(out=r_t[:, :F], in_=hpre[:, :F],
                                         func=mybir.ActivationFunctionType.Relu, scale=ra)
                    nc.vector.tensor_scalar_mul(th_t[:, :F], th_t[:, :F], ta)
                    nc.vector.tensor_scalar(out=e_t[:, :F], in0=e_t[:, :F],
                                            scalar1=ea, scalar2=-ea,
                                            op0=mybir.AluOpType.mult,
                                            op1=mybir.AluOpType.add)
                    nc.vector.tensor_tensor(out=r_t[:, :F], in0=r_t[:, :F],
                                            in1=th_t[:, :F], op=mybir.AluOpType.add)
                    if l == 0:
                        hdst = h1loc_t[:, pr0 * H:pr0 * H + F]
                    else:
                        h2sb = lnpool.tile([P, max(npr, 1) * H], BF, tag="h2")
                        hdst = h2sb[:, :F]
                    nc.vector.tensor_tensor(out=hdst, in0=r_t[:, :F], in1=e_t[:, :F],
                                            op=mybir.AluOpType.add)

                    if l == 0:
                        for prl in range(npr):
                            pr = pr0 + prl
                            rows = min(P, ns - pr * P)
                            if rows > 0:
                                nc.sync.dma_start(
                                    out=h1s_d.ap()[pr * P:pr * P + rows, :],
                                    in_=h1loc_t[0:rows, pr * H:(pr + 1) * H])
                            pt = ps_tr.tile([P, P], BF, tag="tr")
                            nc.tensor.transpose(out=pt[:],
                                                in_=h1loc_t[:, pr * H:(pr + 1) * H],
                                                identity=ident_t[:])
                            nc.vector.tensor_copy(out=h1T_t[:, pr * P:(pr + 1) * P],
                                                  in_=pt[:])
                    else:
                        skip = h2sb
                        nc.vector.tensor_tensor(out=skip[:, :F],
                                                in0=h1loc_t[:, pr0 * H:pr0 * H + F],
                                                in1=hdst, op=mybir.AluOpType.add)
                        for prl in range(npr):
                            pr = pr0 + prl
                            nc.tensor.matmul(
                                pool_psum[:],
                                lhsT=epool_t[:, pr * GSLOTS:(pr + 1) * GSLOTS],
                                rhs=skip[:, prl * H:(prl + 1) * H],
                                start=(pr == 0), stop=(pr == cfg.npair - 1))

            run_layer(0)
            nc.gpsimd.collective_compute(
                "AllGather", mybir.AluOpType.bypass,
                replica_groups=[list(range(cfg.cores))],
                ins=[h1s_d.ap()], outs=[h1f_d.ap()])
            run_layer(1)

            # ---------- readout: pooled @ post_w ----------
            poolc = smpool.tile([GSLOTS, H], BF, tag="poolc")
            nc.vector.tensor_copy(out=poolc[:], in_=pool_psum[:])
            pt = ps_tr.tile([P, GSLOTS], BF, tag="tr")
            nc.tensor.transpose(out=pt[:], in_=poolc[:], identity=ident_t[:])
            ptc = smpool.tile([P, GSLOTS], BF, tag="ptc")
            nc.vector.tensor_copy(out=ptc[:], in_=pt[:])
            ops = ps_dense.tile([GSLOTS, DOUT], F32, tag="dense")
            nc.tensor.matmul(ops[:], lhsT=ptc[:], rhs=pw_t[:], start=True, stop=True)
            outc = smpool.tile([GSLOTS, DOUT], F32, tag="outc")
            nc.vector.tensor_copy(out=outc[:], in_=ops[:])
            nc.sync.dma_start(out=out_d.ap(), in_=outc[:])

    nc.compile()
    return nc


def _kernel_impl(inputs: dict, cfg: Cfg = None, trace: bool = False):
    if cfg is None:
        cfg = Cfg(N=50000, E=640000, G=500, cores=8, half=32768)
    sched, data, combine = host_prep(inputs, cfg)
    nc = build_program(cfg, sched)
    in_maps = [data[c] for c in range(cfg.cores)]
    res = run_bass_kernel_spmd(nc, in_maps, core_ids=list(range(cfg.cores)),
                               trace=trace)
    out = np.zeros((cfg.G, DOUT), np.float64)
    for c in range(cfg.cores):
        part = np.asarray(res.results[c]["out_part"], np.float64)
        lo = combine["g_lo"][c]
        hi = min(lo + GSLOTS, cfg.G)
        out[lo:hi] += part[:hi - lo]
    out += combine["post_b"]
    return out.astype(np.float32), res


def kernel(**inputs) -> np.ndarray:
    out, _ = _kernel_impl(inputs)
    return out
